# revision 22
# baseline (speedup 1.0000x reference)
"""Trainium2 Bass kernel for nn_Attention_89670327206161 (Gram restructure).

The reference contracts attention scores over the *sequence* axis, so per
head the score matrix is only (dh x dh) = 64x64:
    scores_h = K_h^T Q_h / 8 = Wk_h (x^T x) Wq_h^T / 8
    out      = x . Wv^T . blockdiag(softmax(scores)) . Wo^T
The whole layer therefore collapses to GEMMs around one 1024x1024 Gram
matrix instead of three projections + attention + out-projection:
    G  = x^T x          (symmetric: pass A = cols 0:512 all rows,
                         pass B = bottom-right quadrant, top-right
                         quadrant mirrored with PE transposes)
    T2 = G Wq^T         (G's symmetry supplies the lhsT blocks directly)
    scores_p = Wk_pair T2   (256-wide rhs keeps fp32r at full rate)
    P  = softmax_rows(scores/8)  (max-subtracted exp per 64x64 block)
    U^T = BD(P)^T Wv-rows;  M = U Wo^T;  outT = M-blocks^T @ xT
PE work ~620K cycles/core vs ~1.15M for the direct algorithm.

Sharding: pure data parallelism -- one batch element per core, no
collectives. Host supplies x twice (seq-major for G, feature-major for
the final pass); output returns feature-major and is transposed on host.

DMA choreography (the xa stream must never starve):
  sync queue   : xa seq tiles, xb half tiles, xt blocks, out blocks --
                 all x traffic, in consumption order.
  gpsimd queue : wq-half0 (chunked) from t=0; a 1-elem gate copy that
                 depends on the first pass-B copy-out holds wq-half1 and
                 wk (chunked) until pass B's xb stream has the bus to
                 itself; both land during T2's DMA-quiet window.
  scalar queue : wv/wo halves issued right after scalar's T2 copies
                 (before its softmax exps), landing during the scores
                 window just before U^T / M consume them.
SBUF arena: one bufs=1 pool with three 32KB tag rings reused across
phases (g->m, wq->wv, wk->wo) -- the reuse both forces placement and
gives the weight DMAs their anti-dependency timing.
"""

import numpy as np

HEADS = 16
B, S, D = 8, 4096, 1024
P = 128
NKC = D // P             # 8 chunks of 128 along D
NT = S // P              # 32 seq tiles
NPAIR = HEADS // 2       # 8 head pairs -> 128-wide blocks
N_CORES = 8

_PROGRAM = None


def _ts(i, n):
    return slice(i * n, (i + 1) * n)


def _build_program():
    import concourse.bacc as bacc
    import concourse.mybir as mybir
    import concourse.tile as tile
    from concourse.masks import make_identity

    f32 = mybir.dt.float32
    f32r = mybir.dt.float32r
    EXP = mybir.ActivationFunctionType.Exp
    X = mybir.AxisListType.X

    nc = bacc.Bacc(trn_type="TRN2", debug=False, num_devices=N_CORES)

    xs = nc.dram_tensor("xs", [S, D], f32r, kind="ExternalInput")
    xT = nc.dram_tensor("xT", [D, S], f32r, kind="ExternalInput")
    wqT = nc.dram_tensor("wqT", [D, D], f32r, kind="ExternalInput")
    wkT = nc.dram_tensor("wkT", [D, D], f32r, kind="ExternalInput")
    wv = nc.dram_tensor("wv", [D, D], f32r, kind="ExternalInput")
    woT = nc.dram_tensor("woT", [D, D], f32r, kind="ExternalInput")
    outT = nc.dram_tensor("outT", [D, S], f32, kind="ExternalOutput")

    xs_ap = xs.ap()
    xTr = xT.ap().rearrange("(c p) s -> p c s", p=P)
    wqTr = wqT.ap().rearrange("(c p) o -> p c o", p=P)
    wkTr = wkT.ap().rearrange("(c p) o -> p c o", p=P)
    wvr = wv.ap().rearrange("(r p) c -> p r c", p=P)
    woTr = woT.ap().rearrange("(c p) o -> p c o", p=P)
    outTr = outT.ap().rearrange("(c p) s -> p c s", p=P)

    with tile.TileContext(nc) as tc:
      with (
          tc.tile_pool(name="const", bufs=1) as const_pool,
          tc.tile_pool(name="persist", bufs=1) as persist_pool,
          tc.tile_pool(name="smx", bufs=4) as smx_pool,
          tc.tile_pool(name="t2u", bufs=1) as t2u_pool,
          tc.tile_pool(name="arena", bufs=1) as arena,
      ):
        zero_sb = const_pool.tile([P, 512], f32r, tag="zero")
        ident_raw = const_pool.tile([P, P], f32, tag="identr")
        ident = const_pool.tile([P, P], f32r, tag="ident")
        p_all = persist_pool.tile([P, NPAIR, P], f32r, tag="pall")

        nc.vector.memset(zero_sb[:].bitcast(f32), 0.0)
        nc.vector.memset(p_all[:].bitcast(f32), 0.0)
        # affine_select output isn't fp32r-rounded for the BIR verifier;
        # route it through a copy, which is
        make_identity(nc, ident_raw[:])
        nc.vector.tensor_copy(ident[:], ident_raw[:])

        g_sb = arena.tile([P, NKC, D], f32r, tag="a")
        wq_sb = arena.tile([P, NKC, D], f32r, tag="b")
        wk_sb = arena.tile([P, NKC, D], f32r, tag="c")
        t2_sb = t2u_pool.tile([P, NKC, D], f32r, tag="t2")

        with (
            tc.tile_pool(name="xa", bufs=4) as xa_pool,
            tc.tile_pool(name="xbp", bufs=4) as xb_pool,
        ):
            with tc.tile_pool(name="gA", bufs=1, space="PSUM") as gA_pool:
                # ---- G pass A: G[:, 0:512] = sum_st xs_st^T xs_st
                g_ps = gA_pool.tile([P, NKC * 512], f32, tag="gps")
                # HAM warm-up + has_written clear: one dummy per bank
                for ci in range(NKC):
                    nc.tensor.matmul(
                        g_ps[:, _ts(ci, 512)], zero_sb[:, 0:P], zero_sb[:],
                        start=True, stop=False, skip_group_check=True,
                    )
                for st in range(NT):
                    xa = xa_pool.tile([P, D], f32r, tag="xa")
                    nc.sync.dma_start(xa[:], xs_ap[_ts(st, P), :])
                    for ci in range(NKC):
                        nc.tensor.matmul(
                            g_ps[:, _ts(ci, 512)],
                            xa[:, _ts(ci, P)], xa[:, 0:512],
                            start=False, stop=(st == NT - 1),
                            skip_group_check=True,
                        )
                    if st == 26:
                        # wq half0 rides behind tile 20's arrival. The tile
                        # scheduler orders DMAs by data deps only, so the
                        # gate is a WAW hazard: a byte written into wq_sb
                        # (sourced from this xa tile) that every wq0 chunk
                        # DMA must overwrite afterwards.
                        nc.gpsimd.tensor_copy(
                            wq_sb[0:1, 0, 0:512], xa[0:1, 0:512])
                        for ch in range(4):
                            nc.gpsimd.dma_start(
                                wq_sb[:, :, _ts(ch, P)], wqTr[:, :, _ts(ch, P)])
                for ci in range(NKC):
                    eng = nc.scalar.copy if ci % 2 == 0 else nc.vector.tensor_copy
                    eng(g_sb[:, ci, 0:512], g_ps[:, _ts(ci, 512)])

            # ---- G pass B: bottom-right quadrant + TR mirror
            with (
                tc.tile_pool(name="gB", bufs=1, space="PSUM") as gB_pool,
                tc.tile_pool(name="tr", bufs=2, space="PSUM") as tr_pool,
            ):
                gb_ps = gB_pool.tile([P, 4 * 512], f32, tag="gbps")
                for st in range(NT):
                    xb = xb_pool.tile([P, 512], f32r, tag="xb")
                    nc.sync.dma_start(xb[:], xs_ap[_ts(st, P), 512:D])
                    for j in range(4):
                        nc.tensor.matmul(
                            gb_ps[:, _ts(j, 512)],
                            xb[:, _ts(j, P)], xb[:],
                            start=(st == 0), stop=(st == NT - 1),
                            skip_group_check=True,
                        )
                for j in range(4):
                    eng = nc.scalar.copy if j % 2 == 0 else nc.vector.tensor_copy
                    eng(g_sb[:, 4 + j, 512:D], gb_ps[:, _ts(j, 512)])

                # gate: wq half1 + wk wait until pass B's first copy-out
                # lands (WAW bytes written into their dst regions), so the
                # xa/xb streams own the bus through both G passes; both
                # then load during T2's DMA-quiet window
                nc.gpsimd.tensor_copy(
                    wq_sb[0:1, 0, 512:D], g_sb[0:1, 4, 512:D])
                nc.gpsimd.tensor_copy(
                    wk_sb[0:1, 0, :], g_sb[0:1, 4, :])
                for ch in range(4):
                    nc.gpsimd.dma_start(
                        wq_sb[:, :, _ts(4 + ch, P)], wqTr[:, :, _ts(4 + ch, P)])
                for ch in range(NKC):
                    nc.gpsimd.dma_start(
                        wk_sb[:, :, _ts(ch, P)], wkTr[:, :, _ts(ch, P)])

                # TR quadrant: G[k-chunk, col 512+128j block] =
                # (G[(4+j)-chunk, k block])^T  by symmetry
                for j in range(4):
                    for k in range(4):
                        t_ps = tr_pool.tile([P, P], f32r, tag="tr")
                        nc.tensor.transpose(
                            t_ps[:], g_sb[:, 4 + j, _ts(k, P)], ident[:]
                        )
                        eng = nc.scalar.copy if (j + k) % 2 == 0 else nc.vector.tensor_copy
                        eng(g_sb[:, k, 512 + j * P:512 + (j + 1) * P], t_ps[:])

        # ---- T2 / scores / softmax era
        with (
            tc.tile_pool(name="big", bufs=4, space="PSUM") as big_pool,
            tc.tile_pool(name="sc", bufs=4, space="PSUM") as sc_pool,
        ):
            def t2_half(h):
                for m in range(NKC):
                    ps = big_pool.tile([P, 512], f32, tag="bps")
                    for c in range(NKC):
                        nc.tensor.matmul(
                            ps[:], g_sb[:, c, _ts(m, P)],
                            wq_sb[:, c, _ts(h, 512)],
                            start=(c == 0), stop=(c == NKC - 1),
                        )
                    eng = nc.scalar.copy if m % 2 == 0 else nc.vector.tensor_copy
                    eng(t2_sb[:, m, _ts(h, 512)], ps[:])

            t2_half(0)
            t2_half(1)

            # wv/wo reuse wq/wk's arena rings: the allocator pins them to
            # that space and the anti-dependency releases each DMA the
            # moment its predecessor's last read retires. Issued on the
            # scalar queue ahead of the softmax exps.
            wv_sb = arena.tile([P, NKC, D], f32r, tag="b")
            wo_sb = arena.tile([P, NKC, D], f32r, tag="c")
            for h in range(2):
                nc.scalar.dma_start(wv_sb[:, :, _ts(h, 512)], wvr[:, :, _ts(h, 512)])
            for h in range(2):
                nc.scalar.dma_start(wo_sb[:, :, _ts(h, 512)], woTr[:, :, _ts(h, 512)])

            def softmax_half(ps, off, p, hf):
                # scores arrive pre-scaled by 1/8 (folded into the wq
                # upload), and exp's accum_out yields the row sum for
                # free: max -> exp(+sum) -> recip -> mul, 3 engine hops
                rows = slice(64 * hf, 64 * hf + 64)
                cols = slice(off + 64 * hf, off + 64 * hf + 64)
                mx = smx_pool.tile([P, 1], f32, tag="mx")
                nc.vector.reduce_max(mx[rows, 0:1], ps[rows, cols], axis=X, negate=True)
                et = smx_pool.tile([P, 64], f32, tag="et")
                den = smx_pool.tile([P, 1], f32, tag="den")
                rec = smx_pool.tile([P, 1], f32, tag="rec")
                nc.scalar.activation(
                    et[rows, :], ps[rows, cols], EXP,
                    bias=mx[rows, 0:1], accum_out=den[rows, 0:1],
                )
                nc.vector.reciprocal(rec[rows, 0:1], den[rows, 0:1])
                nc.gpsimd.tensor_scalar_mul(
                    p_all[rows, p, 64 * hf:64 * hf + 64],
                    et[rows, :], rec[rows, 0:1],
                )

            for p in range(NPAIR):
                c0 = min(p * P, D - 256)
                ps = sc_pool.tile([P, 256], f32, tag="sc")
                for m in range(NKC):
                    nc.tensor.matmul(
                        ps[:], wk_sb[:, m, _ts(p, P)],
                        t2_sb[:, m, c0:c0 + 256],
                        start=(m == 0), stop=(m == NKC - 1),
                    )
                off = p * P - c0
                softmax_half(ps, off, p, 0)
                softmax_half(ps, off, p, 1)

        # ---- U^T / M / outT era
        ut_sb = t2u_pool.tile([P, NKC, D], f32r, tag="t2")
        m_sb = arena.tile([P, NKC, D], f32r, tag="a")
        with (
            tc.tile_pool(name="xt", bufs=2) as xt_pool,
            tc.tile_pool(name="ot", bufs=2) as ot_pool,
            tc.tile_pool(name="big2", bufs=4, space="PSUM") as big2_pool,
        ):
            # xt0 gated (WAW) on T2h0's copy-out so its 2MB transfer stays
            # out of the G-phase window
            xt0 = xt_pool.tile([P, NKC, 512], f32r, tag="xt")
            nc.gpsimd.tensor_copy(xt0[0:1, 0, 0:1], t2_sb[0:1, 0, 0:1])
            nc.gpsimd.dma_start(xt0[:], xTr[:, :, 0:512])

            # ---- U^T: per pair, U^T[pair-rows] = P_pair^T @ Wv[pair-rows]
            # (h-outer matches the half-split wv arrival order)
            for h in range(2):
                for p in range(NPAIR):
                    ps = big2_pool.tile([P, 512], f32, tag="bps")
                    nc.tensor.matmul(
                        ps[:], p_all[:, p, :], wv_sb[:, p, _ts(h, 512)],
                        start=True, stop=True,
                    )
                    eng = nc.scalar.copy if p % 2 == 0 else nc.vector.tensor_copy
                    eng(ut_sb[:, p, _ts(h, 512)], ps[:])

            # ---- M = U @ Wo^T
            for h in range(2):
                for a in range(NKC):
                    ps = big2_pool.tile([P, 512], f32, tag="bps")
                    for c in range(NKC):
                        nc.tensor.matmul(
                            ps[:], ut_sb[:, c, _ts(a, P)],
                            wo_sb[:, c, _ts(h, 512)],
                            start=(c == 0), stop=(c == NKC - 1),
                        )
                    eng = nc.scalar.copy if a % 2 == 0 else nc.vector.tensor_copy
                    eng(m_sb[:, a, _ts(h, 512)], ps[:])

            # ---- outT = M-blocks^T @ xT, streamed in 512-seq blocks
            for sb in range(NKC):
                if sb == 0:
                    xt = xt0
                else:
                    xt = xt_pool.tile([P, NKC, 512], f32r, tag="xt")
                    nc.sync.dma_start(xt[:], xTr[:, :, _ts(sb, 512)])
                ot = ot_pool.tile([P, NKC, 512], f32, tag="ot")
                for oc in range(NKC):
                    ps = big2_pool.tile([P, 512], f32, tag="bps")
                    for ci in range(NKC):
                        nc.tensor.matmul(
                            ps[:], m_sb[:, ci, _ts(oc, P)], xt[:, ci, :],
                            start=(ci == 0), stop=(ci == NKC - 1),
                        )
                    eng = nc.scalar.copy if oc % 2 == 0 else nc.vector.tensor_copy
                    eng(ot[:, oc, :], ps[:])
                    nc.sync.dma_start(outTr[:, oc, _ts(sb, 512)], ot[:, oc, :])

    nc.compile()
    return nc


def _get_program():
    global _PROGRAM
    if _PROGRAM is None:
        _PROGRAM = _build_program()
    return _PROGRAM


def kernel(x, Wq, Wk, Wv, Wo):
    from concourse import bass_utils

    nc = _get_program()

    x = np.asarray(x, np.float32)
    xs_all = np.ascontiguousarray(x)
    xT_all = np.ascontiguousarray(np.transpose(x, (0, 2, 1)))
    # 1/8 score scale folded into wq so softmax needs no scale pass
    wqT = np.ascontiguousarray(np.asarray(Wq, np.float32).T * np.float32(0.125))
    wkT = np.ascontiguousarray(np.asarray(Wk, np.float32).T)
    wv_ = np.ascontiguousarray(np.asarray(Wv, np.float32))
    woT = np.ascontiguousarray(np.asarray(Wo, np.float32).T)

    in_maps = [
        {"xs": xs_all[b], "xT": xT_all[b], "wqT": wqT, "wkT": wkT,
         "wv": wv_, "woT": woT}
        for b in range(N_CORES)
    ]
    res = bass_utils.run_bass_kernel_spmd(nc, in_maps, core_ids=list(range(N_CORES)))
    outT_all = np.stack([res.results[b]["outT"] for b in range(N_CORES)], axis=0)
    return np.ascontiguousarray(np.transpose(outT_all, (0, 2, 1)))
